# revision 5
# baseline (speedup 1.0000x reference)
"""CenterLoss kernel for Trainium2 (8 NeuronCores, Bass/Tile).

Strategy (class-sharded):
  - centers [100000,128] split into 8 shards of 12500 rows (+1 junk row).
  - Batch items routed on host to the core owning their class, sorted by
    class, packed into 128-item chunks such that no class's run crosses a
    chunk boundary (pad with junk items). All host work is integer index
    bookkeeping on y only (routing/sort/counts -> -alpha/(n+1) factors).
  - Per core the device:
      * bulk-copies its centers shard to the output (dominant HBM traffic)
      * indirect-gathers each chunk's center rows ([128,1] offsets only:
        wider offset APs mis-execute on real HW)
      * per chunk: diff = c - x; loss row-sums via ACT square+accum;
        one-hot(first-occurrence rank) matmul on PE merges duplicate
        classes; new row = c + af * upd on DVE
      * indirect-scatters final rows (unique targets; duplicates aimed at
        the junk row). Scatters drain after the bulk copy (WAW).
  - Host concatenates the 8 output shards and sums the 8 loss partials.

Engine budget per core: ~36 SWDGE indirect ops on GpSimd/Q7 (~45us, the
serial floor alongside ~18MB HBM traffic ~50us); DVE ~3 ops/chunk; PE one
128x128x128 matmul per chunk; ACT one square+accum per chunk.
"""

import numpy as np

import concourse.bass as bass
import concourse.tile as tile
from concourse import bacc, mybir
from concourse import bass_utils

NB_CLASS = 100000
DIM = 128
BATCH = 16384
LOSS_WEIGHT = 0.01
ALPHA = 0.05

NCORES = 8
SHARD = NB_CLASS // NCORES  # 12500
JUNK = SHARD  # junk row index in the per-core shard (extra row)
P = 128  # chunk size == partitions
NCHUNK = 18
NPAD = NCHUNK * P  # 2304
GRP = 6  # chunks per x-load DMA
NGRP = NCHUNK // GRP  # 3
COPY_SLICES = 4  # bulk-copy split across HWDGE queues

FP = mybir.dt.float32
I32 = mybir.dt.int32


def _build_program():
    nc = bacc.Bacc("TRN2", target_bir_lowering=False, debug=False,
                   num_devices=NCORES)

    centers_t = nc.dram_tensor("centers_s", [SHARD + 1, DIM], FP,
                               kind="ExternalInput")
    x_t = nc.dram_tensor("x_s", [NPAD, DIM], FP, kind="ExternalInput")
    lidx_t = nc.dram_tensor("lidx_s", [P, NCHUNK], I32, kind="ExternalInput")
    rank_t = nc.dram_tensor("rank_s", [P, NCHUNK], FP, kind="ExternalInput")
    uslot_t = nc.dram_tensor("uslot_s", [P, NCHUNK], I32, kind="ExternalInput")
    af_t = nc.dram_tensor("af_s", [P, NCHUNK], FP, kind="ExternalInput")

    newc_t = nc.dram_tensor("newc_s", [SHARD + 1, DIM], FP,
                            kind="ExternalOutput")
    loss_t = nc.dram_tensor("loss_s", [1, 1], FP, kind="ExternalOutput")

    centers_ap = centers_t.ap()
    newc_ap = newc_t.ap()

    with tile.TileContext(nc) as tc:
        with tc.tile_pool(name="const", bufs=1) as cpool, \
             tc.tile_pool(name="gat", bufs=NCHUNK) as gpool, \
             tc.tile_pool(name="out", bufs=NCHUNK) as opool, \
             tc.tile_pool(name="xs", bufs=NGRP) as xpool, \
             tc.tile_pool(name="work", bufs=6) as wpool, \
             tc.tile_pool(name="psum", bufs=6, space="PSUM") as ppool, \
             tc.tile_pool(name="psl", bufs=1, space="PSUM") as plpool:

            # ---- one-time constants ----
            iota_i = cpool.tile([P, P], I32)
            nc.gpsimd.iota(iota_i[:], pattern=[[1, P]], base=0,
                           channel_multiplier=0)
            iota_f = cpool.tile([P, P], FP)
            nc.vector.tensor_copy(iota_f[:], iota_i[:])
            ones_col = cpool.tile([P, 1], FP)
            nc.vector.memset(ones_col[:], 1.0)
            sacc_all = cpool.tile([P, NCHUNK], FP)

            # ---- bulk copy centers shard -> output shard (DRAM->DRAM) ----
            rows = SHARD // COPY_SLICES
            for s in range(COPY_SLICES):
                eng = nc.sync if s % 2 == 0 else nc.scalar
                r0, r1 = s * rows, (s + 1) * rows
                eng.dma_start(newc_ap[r0:r1, :], centers_ap[r0:r1, :])

            # ---- metadata (single DMAs) ----
            lidx_m = cpool.tile([P, NCHUNK], I32)
            nc.sync.dma_start(lidx_m[:], lidx_t.ap()[:, :])
            rank_m = cpool.tile([P, NCHUNK], FP)
            nc.scalar.dma_start(rank_m[:], rank_t.ap()[:, :])
            uslot_m = cpool.tile([P, NCHUNK], I32)
            nc.sync.dma_start(uslot_m[:], uslot_t.ap()[:, :])
            af_m = cpool.tile([P, NCHUNK], FP)
            nc.scalar.dma_start(af_m[:], af_t.ap()[:, :])

            # ---- x loads (batched across chunks) ----
            xgs = []
            for g in range(NGRP):
                c0 = g * GRP
                xg = xpool.tile([P, GRP, DIM], FP, tag="xg")
                eng = nc.sync if g % 2 == 0 else nc.scalar
                eng.dma_start(
                    xg[:],
                    x_t.ap()[c0 * P:(c0 + GRP) * P, :]
                       .rearrange("(c p) d -> p c d", p=P))
                xgs.append(xg)

            # ---- all gathers first: keeps Q7 busy under the bulk copy ----
            cgs = []
            for c in range(NCHUNK):
                cg = gpool.tile([P, DIM], FP, tag="cg")
                nc.gpsimd.indirect_dma_start(
                    out=cg[:],
                    out_offset=None,
                    in_=centers_ap[:, :],
                    in_offset=bass.IndirectOffsetOnAxis(
                        ap=lidx_m[:, c:c + 1], axis=0))
                cgs.append(cg)

            # ---- per-chunk compute ----
            outcs = []
            for c in range(NCHUNK):
                cg = cgs[c]
                xg = xgs[c // GRP]
                cc = c % GRP

                diff = wpool.tile([P, DIM], FP, tag="diff")
                nc.vector.tensor_sub(diff[:], cg[:], xg[:, cc, :])

                # loss: ACT square + free-axis accumulate into column c
                sq = wpool.tile([P, DIM], FP, tag="sq")
                nc.scalar.activation(
                    out=sq[:], in_=diff[:],
                    func=mybir.ActivationFunctionType.Square,
                    accum_out=sacc_all[:, c:c + 1])

                # one-hot of first-occurrence rank
                onehot = wpool.tile([P, P], FP, tag="onehot")
                nc.vector.tensor_tensor(
                    out=onehot[:],
                    in0=rank_m[:, c:c + 1].to_broadcast([P, P]),
                    in1=iota_f[:],
                    op=mybir.AluOpType.is_equal)

                # upd[slot, :] = sum of diff rows sharing the slot's class
                ps = ppool.tile([P, DIM], FP, tag="ps")
                nc.tensor.matmul(out=ps[:], lhsT=onehot[:], rhs=diff[:],
                                 start=True, stop=True)

                # new row = c + af * upd   (af = -alpha/(count+1))
                outc = opool.tile([P, DIM], FP, tag="outc")
                nc.vector.tensor_scalar(
                    out=outc[:], in0=ps[:],
                    scalar1=af_m[:, c:c + 1], scalar2=None,
                    op0=mybir.AluOpType.mult)
                nc.vector.tensor_add(outc[:], outc[:], cg[:])
                outcs.append(outc)

            # ---- scatters drain after the bulk copy (WAW on newc) ----
            for c in range(NCHUNK):
                # bounds_check + oob_is_err=False is required: the
                # no-bounds-regs indirect-scatter ucode wedges on HW.
                nc.gpsimd.indirect_dma_start(
                    out=newc_ap[:, :],
                    out_offset=bass.IndirectOffsetOnAxis(
                        ap=uslot_m[:, c:c + 1], axis=0),
                    in_=outcs[c][:],
                    in_offset=None,
                    bounds_check=SHARD,
                    oob_is_err=False)

            # ---- loss: reduce columns, cross-partition sum, scale ----
            lacc = cpool.tile([P, 1], FP)
            nc.vector.tensor_reduce(out=lacc[:], in_=sacc_all[:],
                                    axis=mybir.AxisListType.X,
                                    op=mybir.AluOpType.add)
            psl = plpool.tile([1, 1], FP)
            nc.tensor.matmul(out=psl[:], lhsT=lacc[:], rhs=ones_col[:],
                             start=True, stop=True)
            loss_sb = cpool.tile([1, 1], FP)
            nc.vector.tensor_scalar_mul(loss_sb[:], psl[:],
                                        LOSS_WEIGHT / BATCH)
            nc.sync.dma_start(loss_t.ap()[:, :], loss_sb[:])

    nc.compile()
    return nc


_NC = None


def _get_program():
    global _NC
    if _NC is None:
        _NC = _build_program()
    return _NC


def _pack_core(cls_loc: np.ndarray, x_core: np.ndarray):
    """Pack one core's sorted items into chunks of P with no class run
    crossing a chunk boundary. Returns device input arrays."""
    m = cls_loc.shape[0]
    if m == 0:
        starts = np.zeros(0, np.int64)
        lens = np.zeros(0, np.int64)
    else:
        starts = np.flatnonzero(np.r_[True, cls_loc[1:] != cls_loc[:-1]])
        lens = np.diff(np.r_[starts, m])

    place = np.empty(len(starts), np.int64)
    pos = 0
    for i, L in enumerate(lens):
        room = P - (pos % P)
        if L > room:
            pos += room
        assert L <= P, f"class run of length {L} exceeds chunk size"
        place[i] = pos
        pos += L
    assert pos <= NPAD, f"core needs {pos} slots > NPAD={NPAD}"

    # per-item output position
    out_pos = np.repeat(place, lens) + (np.arange(m) - np.repeat(starts, lens))

    xk = np.zeros((NPAD, DIM), np.float32)
    xk[out_pos] = x_core
    lidx = np.full(NPAD, JUNK, np.int32)
    lidx[out_pos] = cls_loc
    rank = (np.arange(NPAD) % P).astype(np.float32)
    rank[out_pos] = np.repeat((place % P).astype(np.float32), lens)
    uslot = np.full(NPAD, JUNK, np.int32)
    uslot[place] = cls_loc[starts]
    af = np.zeros(NPAD, np.float32)
    af[place] = -ALPHA / (lens + 1.0).astype(np.float32)

    def cols(a):
        return np.ascontiguousarray(a.reshape(NCHUNK, P).T)

    return {
        "x_s": xk,
        "lidx_s": cols(lidx),
        "rank_s": cols(rank.astype(np.float32)),
        "uslot_s": cols(uslot),
        "af_s": cols(af.astype(np.float32)),
    }


def make_in_maps(x: np.ndarray, y: np.ndarray, centers: np.ndarray):
    order = np.argsort(y, kind="stable")
    ys = y[order]
    xs = x[order]
    bounds = np.searchsorted(ys, np.arange(NCORES + 1) * SHARD)

    in_maps = []
    for k in range(NCORES):
        lo, hi = bounds[k], bounds[k + 1]
        im = _pack_core((ys[lo:hi] - k * SHARD).astype(np.int64), xs[lo:hi])
        shard = np.empty((SHARD + 1, DIM), np.float32)
        shard[:SHARD] = centers[k * SHARD:(k + 1) * SHARD]
        shard[SHARD] = 0.0
        im["centers_s"] = shard
        in_maps.append(im)
    return in_maps


LAST_RESULTS = None


def kernel(x: np.ndarray, y: np.ndarray, centers: np.ndarray):
    global LAST_RESULTS
    x = np.ascontiguousarray(np.asarray(x, np.float32))
    y = np.asarray(y, np.int32)
    centers = np.ascontiguousarray(np.asarray(centers, np.float32))

    in_maps = make_in_maps(x, y, centers)
    nc = _get_program()
    res = bass_utils.run_bass_kernel_spmd(nc, in_maps,
                                          core_ids=list(range(NCORES)))
    LAST_RESULTS = res

    new_centers = np.concatenate(
        [res.results[k]["newc_s"][:SHARD] for k in range(NCORES)], axis=0)
    loss = np.float32(sum(float(res.results[k]["loss_s"][0, 0])
                          for k in range(NCORES)))
    return loss, new_centers


# revision 6
# speedup vs baseline: 1.3563x; 1.3563x over previous
"""CenterLoss kernel for Trainium2 (8 NeuronCores, Bass/Tile).

Strategy (class-sharded):
  - centers [100000,128] split into 8 shards of 12500 rows (+1 junk row).
  - Batch items routed on host to the core owning their class, sorted by
    class, packed into 128-item chunks such that no class's run crosses a
    chunk boundary (pad with junk items). All host work is integer index
    bookkeeping on y only (routing/sort/counts -> -alpha/(n+1) factors).
  - The output shard is split into 4 quarter tensors (3125 rows + junk
    row each) and chunks are packed quarter-aligned (5 chunks/quarter):
    scatters to different quarters don't false-WAW-serialize on each
    other, and each scatter only waits for its own quarter's bulk copy.
  - Per core the device:
      * bulk-copies its centers shard to the 4 output quarters
        (dominant HBM traffic), with metadata/x DMAs queued ahead of the
        copies on the HWDGE rings so the gathers can start immediately
      * indirect-gathers each chunk's center rows ([128,1] offsets only:
        wider offset APs mis-execute on real HW)
      * per chunk: diff = c - x; loss row-sums via ACT square+accum;
        one-hot(first-occurrence rank) matmul on PE merges duplicate
        classes; new row = c + af * upd on DVE
      * indirect-scatters final rows (unique targets; duplicates aimed
        at the junk row), interleaved across quarters.
  - Host concatenates the 8x4 output quarters and sums the loss partials.
"""

import numpy as np

import concourse.bass as bass
import concourse.tile as tile
from concourse import bacc, mybir
from concourse import bass_utils

NB_CLASS = 100000
DIM = 128
BATCH = 16384
LOSS_WEIGHT = 0.01
ALPHA = 0.05

NCORES = 8
SHARD = NB_CLASS // NCORES  # 12500
NQ = 4  # output quarters per shard
QROWS = SHARD // NQ  # 3125
QJUNK = QROWS  # junk row index within a quarter tensor
GJUNK = SHARD  # junk row index in the gather source (full shard + 1)
P = 128  # chunk size == partitions
CPQ = 5  # chunks per quarter
NCHUNK = NQ * CPQ  # 20
NPAD = NCHUNK * P  # 2560
GRP = 5  # chunks per x-load DMA
NGRP = NCHUNK // GRP  # 4

FP = mybir.dt.float32
I32 = mybir.dt.int32

OUT_NAMES = [f"newq{q}_s" for q in range(NQ)]


def _build_program():
    nc = bacc.Bacc("TRN2", target_bir_lowering=False, debug=False,
                   num_devices=NCORES)

    centers_t = nc.dram_tensor("centers_s", [SHARD + 1, DIM], FP,
                               kind="ExternalInput")
    x_t = nc.dram_tensor("x_s", [NPAD, DIM], FP, kind="ExternalInput")
    lidx_t = nc.dram_tensor("lidx_s", [P, NCHUNK], I32, kind="ExternalInput")
    rank_t = nc.dram_tensor("rank_s", [P, NCHUNK], FP, kind="ExternalInput")
    uslot_t = nc.dram_tensor("uslot_s", [P, NCHUNK], I32, kind="ExternalInput")
    af_t = nc.dram_tensor("af_s", [P, NCHUNK], FP, kind="ExternalInput")

    newq_t = [nc.dram_tensor(OUT_NAMES[q], [QROWS + 1, DIM], FP,
                             kind="ExternalOutput") for q in range(NQ)]
    loss_t = nc.dram_tensor("loss_s", [1, 1], FP, kind="ExternalOutput")

    centers_ap = centers_t.ap()

    with tile.TileContext(nc) as tc:
        with tc.tile_pool(name="const", bufs=1) as cpool, \
             tc.tile_pool(name="gat", bufs=NCHUNK) as gpool, \
             tc.tile_pool(name="out", bufs=NCHUNK) as opool, \
             tc.tile_pool(name="xs", bufs=NGRP) as xpool, \
             tc.tile_pool(name="work", bufs=6) as wpool, \
             tc.tile_pool(name="psum", bufs=6, space="PSUM") as ppool, \
             tc.tile_pool(name="psl", bufs=1, space="PSUM") as plpool:

            # ---- one-time constants ----
            iota_i = cpool.tile([P, P], I32)
            nc.gpsimd.iota(iota_i[:], pattern=[[1, P]], base=0,
                           channel_multiplier=0)
            iota_f = cpool.tile([P, P], FP)
            nc.vector.tensor_copy(iota_f[:], iota_i[:])
            ones_col = cpool.tile([P, 1], FP)
            nc.vector.memset(ones_col[:], 1.0)
            sacc_all = cpool.tile([P, NCHUNK], FP)

            # ---- metadata first: the gathers need lidx immediately ----
            lidx_m = cpool.tile([P, NCHUNK], I32)
            nc.sync.dma_start(lidx_m[:], lidx_t.ap()[:, :])
            uslot_m = cpool.tile([P, NCHUNK], I32)
            nc.sync.dma_start(uslot_m[:], uslot_t.ap()[:, :])
            rank_m = cpool.tile([P, NCHUNK], FP)
            nc.scalar.dma_start(rank_m[:], rank_t.ap()[:, :])
            af_m = cpool.tile([P, NCHUNK], FP)
            nc.scalar.dma_start(af_m[:], af_t.ap()[:, :])

            # ---- x loads, then bulk copies, interleaved on both rings ----
            # ring order (FIFO per issuing engine):
            #   sync:   lidx, uslot, xg0, copy q0, xg2, copy q1
            #   scalar: rank, af,   xg1, copy q2, xg3, copy q3
            xgs = [None] * NGRP

            def load_xg(g, eng):
                xg = xpool.tile([P, GRP, DIM], FP, tag="xg", name=f"xg{g}")
                eng.dma_start(
                    xg[:],
                    x_t.ap()[g * GRP * P:(g + 1) * GRP * P, :]
                       .rearrange("(c p) d -> p c d", p=P))
                xgs[g] = xg

            def copy_q(q, eng):
                eng.dma_start(newq_t[q].ap()[:QROWS, :],
                              centers_ap[q * QROWS:(q + 1) * QROWS, :])

            load_xg(0, nc.sync)
            copy_q(0, nc.sync)
            load_xg(1, nc.scalar)
            copy_q(2, nc.scalar)
            load_xg(2, nc.sync)
            copy_q(1, nc.sync)
            load_xg(3, nc.scalar)
            copy_q(3, nc.scalar)

            # ---- all gathers first: keeps Q7 busy under the bulk copy ----
            cgs = []
            for c in range(NCHUNK):
                cg = gpool.tile([P, DIM], FP, tag="cg", name=f"cg{c}")
                nc.gpsimd.indirect_dma_start(
                    out=cg[:],
                    out_offset=None,
                    in_=centers_ap[:, :],
                    in_offset=bass.IndirectOffsetOnAxis(
                        ap=lidx_m[:, c:c + 1], axis=0))
                cgs.append(cg)

            # ---- per-chunk compute ----
            outcs = []
            for c in range(NCHUNK):
                cg = cgs[c]
                xg = xgs[c // GRP]
                cc = c % GRP

                diff = wpool.tile([P, DIM], FP, tag="diff", name=f"diff{c}")
                nc.vector.tensor_sub(diff[:], cg[:], xg[:, cc, :])

                # loss: ACT square + free-axis accumulate into column c
                sq = wpool.tile([P, DIM], FP, tag="sq", name=f"sq{c}")
                nc.scalar.activation(
                    out=sq[:], in_=diff[:],
                    func=mybir.ActivationFunctionType.Square,
                    accum_out=sacc_all[:, c:c + 1])

                # one-hot of first-occurrence rank
                onehot = wpool.tile([P, P], FP, tag="onehot", name=f"oh{c}")
                nc.vector.tensor_tensor(
                    out=onehot[:],
                    in0=rank_m[:, c:c + 1].to_broadcast([P, P]),
                    in1=iota_f[:],
                    op=mybir.AluOpType.is_equal)

                # upd[slot, :] = sum of diff rows sharing the slot's class
                ps = ppool.tile([P, DIM], FP, tag="ps", name=f"ps{c}")
                nc.tensor.matmul(out=ps[:], lhsT=onehot[:], rhs=diff[:],
                                 start=True, stop=True)

                # new row = c + af * upd   (af = -alpha/(count+1))
                outc = opool.tile([P, DIM], FP, tag="outc", name=f"outc{c}")
                nc.vector.tensor_scalar(
                    out=outc[:], in0=ps[:],
                    scalar1=af_m[:, c:c + 1], scalar2=None,
                    op0=mybir.AluOpType.mult)
                nc.vector.tensor_add(outc[:], outc[:], cg[:])
                outcs.append(outc)

            # ---- scatters: quarters whose copy lands first go first, and
            # interleave across quarters so the per-tensor WAW completion
            # chains overlap ----
            def scatter(c):
                q = c // CPQ
                # bounds_check + oob_is_err=False is required: the
                # no-bounds-regs indirect-scatter ucode wedges on HW.
                nc.gpsimd.indirect_dma_start(
                    out=newq_t[q].ap()[:, :],
                    out_offset=bass.IndirectOffsetOnAxis(
                        ap=uslot_m[:, c:c + 1], axis=0),
                    in_=outcs[c][:],
                    in_offset=None,
                    bounds_check=QROWS,
                    oob_is_err=False)

            for j in range(CPQ):  # quarters 0 and 2 (copies finish first)
                scatter(0 * CPQ + j)
                scatter(2 * CPQ + j)
            for j in range(CPQ):  # quarters 1 and 3
                scatter(1 * CPQ + j)
                scatter(3 * CPQ + j)

            # ---- loss: reduce columns, cross-partition sum, scale ----
            lacc = cpool.tile([P, 1], FP)
            nc.vector.tensor_reduce(out=lacc[:], in_=sacc_all[:],
                                    axis=mybir.AxisListType.X,
                                    op=mybir.AluOpType.add)
            psl = plpool.tile([1, 1], FP)
            nc.tensor.matmul(out=psl[:], lhsT=lacc[:], rhs=ones_col[:],
                             start=True, stop=True)
            loss_sb = cpool.tile([1, 1], FP)
            nc.vector.tensor_scalar_mul(loss_sb[:], psl[:],
                                        LOSS_WEIGHT / BATCH)
            nc.sync.dma_start(loss_t.ap()[:, :], loss_sb[:])

    nc.compile()
    return nc


_NC = None


def _get_program():
    global _NC
    if _NC is None:
        _NC = _build_program()
    return _NC


def _pack_core(cls_loc: np.ndarray, x_core: np.ndarray):
    """Pack one core's sorted items into quarter-aligned chunks of P with
    no class run crossing a chunk boundary. Returns device input arrays."""
    m = cls_loc.shape[0]

    xk = np.zeros((NPAD, DIM), np.float32)
    lidx = np.full(NPAD, GJUNK, np.int32)
    rank = (np.arange(NPAD) % P).astype(np.float32)
    uslot = np.full(NPAD, QJUNK, np.int32)
    af = np.zeros(NPAD, np.float32)

    qstart = np.searchsorted(cls_loc, np.arange(NQ + 1) * QROWS)
    for q in range(NQ):
        lo, hi = qstart[q], qstart[q + 1]
        mq = hi - lo
        if mq == 0:
            continue
        c = cls_loc[lo:hi]
        xq = x_core[lo:hi]
        starts = np.flatnonzero(np.r_[True, c[1:] != c[:-1]])
        lens = np.diff(np.r_[starts, mq])

        base = q * CPQ * P
        place = np.empty(len(starts), np.int64)
        pos = 0
        for i, L in enumerate(lens):
            room = P - (pos % P)
            if L > room:
                pos += room
            assert L <= P, f"class run of length {L} exceeds chunk size"
            place[i] = pos
            pos += L
        assert pos <= CPQ * P, f"quarter needs {pos} slots > {CPQ * P}"

        out_pos = base + np.repeat(place, lens) + (
            np.arange(mq) - np.repeat(starts, lens))
        xk[out_pos] = xq
        lidx[out_pos] = c  # global row in the shard (for the gather)
        rank[out_pos] = np.repeat((place % P).astype(np.float32), lens)
        uslot[base + place] = c[starts] - q * QROWS  # quarter-local row
        af[base + place] = -ALPHA / (lens + 1.0).astype(np.float32)

    def cols(a):
        return np.ascontiguousarray(a.reshape(NCHUNK, P).T)

    return {
        "x_s": xk,
        "lidx_s": cols(lidx),
        "rank_s": cols(rank.astype(np.float32)),
        "uslot_s": cols(uslot),
        "af_s": cols(af.astype(np.float32)),
    }


def make_in_maps(x: np.ndarray, y: np.ndarray, centers: np.ndarray):
    order = np.argsort(y, kind="stable")
    ys = y[order]
    xs = x[order]
    bounds = np.searchsorted(ys, np.arange(NCORES + 1) * SHARD)

    in_maps = []
    for k in range(NCORES):
        lo, hi = bounds[k], bounds[k + 1]
        im = _pack_core((ys[lo:hi] - k * SHARD).astype(np.int64), xs[lo:hi])
        shard = np.empty((SHARD + 1, DIM), np.float32)
        shard[:SHARD] = centers[k * SHARD:(k + 1) * SHARD]
        shard[SHARD] = 0.0
        im["centers_s"] = shard
        in_maps.append(im)
    return in_maps


def assemble(results):
    new_centers = np.concatenate(
        [results[k][name][:QROWS]
         for k in range(NCORES) for name in OUT_NAMES], axis=0)
    loss = np.float32(sum(float(results[k]["loss_s"][0, 0])
                          for k in range(NCORES)))
    return loss, new_centers


LAST_RESULTS = None


def kernel(x: np.ndarray, y: np.ndarray, centers: np.ndarray):
    global LAST_RESULTS
    x = np.ascontiguousarray(np.asarray(x, np.float32))
    y = np.asarray(y, np.int32)
    centers = np.ascontiguousarray(np.asarray(centers, np.float32))

    in_maps = make_in_maps(x, y, centers)
    nc = _get_program()
    res = bass_utils.run_bass_kernel_spmd(nc, in_maps,
                                          core_ids=list(range(NCORES)))
    LAST_RESULTS = res
    return assemble(res.results)


# revision 7
# speedup vs baseline: 1.4821x; 1.0928x over previous
"""CenterLoss kernel for Trainium2 (8 NeuronCores, Bass/Tile).

Strategy (class-sharded):
  - centers [100000,128] split into 8 shards of 12500 rows (+1 junk row).
  - Batch items routed on host to the core owning their class, sorted by
    class, packed into 128-item chunks such that no class's run crosses a
    chunk boundary (pad with junk items). All host work is integer index
    bookkeeping on y only (routing/sort/counts -> -alpha/(n+1) factors).
  - The output shard is split into 4 quarter tensors (3125 rows each):
    scatters to different quarters don't false-WAW-serialize on each
    other, and each scatter only waits for its own quarter's bulk copy.
    Chunk capacity per quarter slot is static [5,5,5,4]; the host
    permutes the shard's quarters per core (biggest item load first) so
    every quarter fits its slot. 19 chunks -> 38 SWDGE indirect ops.
  - Per core the device:
      * bulk-copies its (permuted) centers shard to the 4 output
        quarters (dominant HBM traffic), with metadata/x DMAs queued
        ahead of the copies on the HWDGE rings
      * indirect-gathers each chunk's center rows ([128,1] offsets only:
        wider offset APs mis-execute on real HW), emitted before
        anything else on GpSimd so descgen hides under the copies
      * per chunk: diff = c - x; loss row-sums via ACT square+accum;
        one-hot(first-occurrence rank) matmul on PE merges duplicate
        classes; new row = c + af * upd on DVE
      * indirect-scatters final rows. Only first-occurrence rows are
        written: duplicate/junk slots carry an out-of-bounds target and
        bounds_check drops those descriptors. Scatter emission is
        interleaved across quarters to overlap completion chains.
  - Host concatenates the 8x4 output quarters (undoing the permutation)
    and sums the 8 loss partials.
"""

import numpy as np

import concourse.bass as bass
import concourse.tile as tile
from concourse import bacc, mybir
from concourse import bass_utils

NB_CLASS = 100000
DIM = 128
BATCH = 16384
LOSS_WEIGHT = 0.01
ALPHA = 0.05

NCORES = 8
SHARD = NB_CLASS // NCORES  # 12500
NQ = 4  # output quarters per shard
QROWS = SHARD // NQ  # 3125
OOB = QROWS + 7  # scatter target for dropped (dup/junk) rows
GJUNK = SHARD  # junk row index in the gather source (full shard + 1)
P = 128  # chunk size == partitions
CAPS = [5, 5, 5, 4]  # chunks per quarter slot
BASE = [0, 5, 10, 15]
NCHUNK = sum(CAPS)  # 19
NPAD = NCHUNK * P  # 2432

FP = mybir.dt.float32
I32 = mybir.dt.int32

OUT_NAMES = [f"newq{q}_s" for q in range(NQ)]


def _build_program():
    nc = bacc.Bacc("TRN2", target_bir_lowering=False, debug=False,
                   num_devices=NCORES)

    centers_t = nc.dram_tensor("centers_s", [SHARD + 1, DIM], FP,
                               kind="ExternalInput")
    x_t = nc.dram_tensor("x_s", [NPAD, DIM], FP, kind="ExternalInput")
    lidx_t = nc.dram_tensor("lidx_s", [P, NCHUNK], I32, kind="ExternalInput")
    rank_t = nc.dram_tensor("rank_s", [P, NCHUNK], FP, kind="ExternalInput")
    uslot_t = nc.dram_tensor("uslot_s", [P, NCHUNK], I32, kind="ExternalInput")
    af_t = nc.dram_tensor("af_s", [P, NCHUNK], FP, kind="ExternalInput")

    newq_t = [nc.dram_tensor(OUT_NAMES[q], [QROWS, DIM], FP,
                             kind="ExternalOutput") for q in range(NQ)]
    loss_t = nc.dram_tensor("loss_s", [1, 1], FP, kind="ExternalOutput")

    centers_ap = centers_t.ap()

    with tile.TileContext(nc) as tc:
        with tc.tile_pool(name="const", bufs=1) as cpool, \
             tc.tile_pool(name="gat", bufs=NCHUNK) as gpool, \
             tc.tile_pool(name="out", bufs=NCHUNK) as opool, \
             tc.tile_pool(name="xs", bufs=NQ) as xpool, \
             tc.tile_pool(name="work", bufs=6) as wpool, \
             tc.tile_pool(name="psum", bufs=6, space="PSUM") as ppool, \
             tc.tile_pool(name="psl", bufs=1, space="PSUM") as plpool:

            # ---- metadata first: the gathers need lidx immediately ----
            lidx_m = cpool.tile([P, NCHUNK], I32)
            nc.sync.dma_start(lidx_m[:], lidx_t.ap()[:, :])
            uslot_m = cpool.tile([P, NCHUNK], I32)
            nc.sync.dma_start(uslot_m[:], uslot_t.ap()[:, :])
            rank_m = cpool.tile([P, NCHUNK], FP)
            nc.scalar.dma_start(rank_m[:], rank_t.ap()[:, :])
            af_m = cpool.tile([P, NCHUNK], FP)
            nc.scalar.dma_start(af_m[:], af_t.ap()[:, :])

            # ---- all gathers first: keeps Q7 busy under the bulk copy ----
            cgs = []
            for c in range(NCHUNK):
                cg = gpool.tile([P, DIM], FP, tag="cg", name=f"cg{c}")
                nc.gpsimd.indirect_dma_start(
                    out=cg[:],
                    out_offset=None,
                    in_=centers_ap[:, :],
                    in_offset=bass.IndirectOffsetOnAxis(
                        ap=lidx_m[:, c:c + 1], axis=0))
                cgs.append(cg)

            # ---- x loads, then bulk copies, interleaved on both rings ----
            # ring order (FIFO per issuing engine):
            #   sync:   lidx, uslot, xg0, copy s0, xg1, copy s1
            #   scalar: rank, af,   xg2, copy s2, xg3, copy s3
            xgs = [None] * NQ

            def load_xg(s, eng):
                xg = xpool.tile([P, CAPS[s], DIM], FP, tag="xg", name=f"xg{s}")
                eng.dma_start(
                    xg[:],
                    x_t.ap()[BASE[s] * P:(BASE[s] + CAPS[s]) * P, :]
                       .rearrange("(c p) d -> p c d", p=P))
                xgs[s] = xg

            def copy_q(q, eng):
                eng.dma_start(newq_t[q].ap()[:, :],
                              centers_ap[q * QROWS:(q + 1) * QROWS, :])

            load_xg(0, nc.sync)
            copy_q(0, nc.sync)
            load_xg(2, nc.scalar)
            copy_q(2, nc.scalar)
            load_xg(1, nc.sync)
            copy_q(1, nc.sync)
            load_xg(3, nc.scalar)
            copy_q(3, nc.scalar)

            # ---- constants for compute ----
            iota_i = cpool.tile([P, P], I32)
            nc.gpsimd.iota(iota_i[:], pattern=[[1, P]], base=0,
                           channel_multiplier=0)
            iota_f = cpool.tile([P, P], FP)
            nc.vector.tensor_copy(iota_f[:], iota_i[:])
            ones_col = cpool.tile([P, 1], FP)
            nc.vector.memset(ones_col[:], 1.0)
            sacc_all = cpool.tile([P, NCHUNK], FP)

            # ---- per-chunk compute ----
            def slot_of(c):
                for s in range(NQ):
                    if c < BASE[s] + CAPS[s]:
                        return s
                raise ValueError(c)

            outcs = []
            for c in range(NCHUNK):
                cg = cgs[c]
                s = slot_of(c)
                xg = xgs[s]
                cc = c - BASE[s]

                diff = wpool.tile([P, DIM], FP, tag="diff", name=f"diff{c}")
                nc.vector.tensor_sub(diff[:], cg[:], xg[:, cc, :])

                # loss: ACT square + free-axis accumulate into column c
                sq = wpool.tile([P, DIM], FP, tag="sq", name=f"sq{c}")
                nc.scalar.activation(
                    out=sq[:], in_=diff[:],
                    func=mybir.ActivationFunctionType.Square,
                    accum_out=sacc_all[:, c:c + 1])

                # one-hot of first-occurrence rank
                onehot = wpool.tile([P, P], FP, tag="onehot", name=f"oh{c}")
                nc.vector.tensor_tensor(
                    out=onehot[:],
                    in0=rank_m[:, c:c + 1].to_broadcast([P, P]),
                    in1=iota_f[:],
                    op=mybir.AluOpType.is_equal)

                # upd[slot, :] = sum of diff rows sharing the slot's class
                ps = ppool.tile([P, DIM], FP, tag="ps", name=f"ps{c}")
                nc.tensor.matmul(out=ps[:], lhsT=onehot[:], rhs=diff[:],
                                 start=True, stop=True)

                # new row = c + af * upd   (af = -alpha/(count+1))
                outc = opool.tile([P, DIM], FP, tag="outc", name=f"outc{c}")
                nc.vector.tensor_scalar(
                    out=outc[:], in0=ps[:],
                    scalar1=af_m[:, c:c + 1], scalar2=None,
                    op0=mybir.AluOpType.mult)
                nc.vector.tensor_add(outc[:], outc[:], cg[:])
                outcs.append(outc)

            # ---- scatters: quarters whose copy lands first go first, and
            # interleave across quarters so the per-tensor WAW completion
            # chains overlap. Dup/junk rows carry an OOB target and are
            # dropped by the bounds check. ----
            def scatter(c):
                s = slot_of(c)
                # bounds_check + oob_is_err=False is required anyway: the
                # no-bounds-regs indirect-scatter ucode wedges on HW.
                nc.gpsimd.indirect_dma_start(
                    out=newq_t[s].ap()[:, :],
                    out_offset=bass.IndirectOffsetOnAxis(
                        ap=uslot_m[:, c:c + 1], axis=0),
                    in_=outcs[c][:],
                    in_offset=None,
                    bounds_check=QROWS - 1,
                    oob_is_err=False)

            for j in range(CAPS[0]):  # slots 0 and 2 (copies finish first)
                scatter(BASE[0] + j)
                if j < CAPS[2]:
                    scatter(BASE[2] + j)
            for j in range(CAPS[1]):  # slots 1 and 3
                scatter(BASE[1] + j)
                if j < CAPS[3]:
                    scatter(BASE[3] + j)

            # ---- loss: reduce columns, cross-partition sum, scale ----
            lacc = cpool.tile([P, 1], FP)
            nc.vector.tensor_reduce(out=lacc[:], in_=sacc_all[:],
                                    axis=mybir.AxisListType.X,
                                    op=mybir.AluOpType.add)
            psl = plpool.tile([1, 1], FP)
            nc.tensor.matmul(out=psl[:], lhsT=lacc[:], rhs=ones_col[:],
                             start=True, stop=True)
            loss_sb = cpool.tile([1, 1], FP)
            nc.vector.tensor_scalar_mul(loss_sb[:], psl[:],
                                        LOSS_WEIGHT / BATCH)
            nc.sync.dma_start(loss_t.ap()[:, :], loss_sb[:])

    nc.compile()
    return nc


_NC = None


def _get_program():
    global _NC
    if _NC is None:
        _NC = _build_program()
    return _NC


def _quarter_pack(c: np.ndarray):
    """Greedy chunk packing of one quarter's sorted local classes.
    Returns (place, lens, starts, packed_size)."""
    mq = c.shape[0]
    if mq == 0:
        z = np.zeros(0, np.int64)
        return z, z, z, 0
    starts = np.flatnonzero(np.r_[True, c[1:] != c[:-1]])
    lens = np.diff(np.r_[starts, mq])
    place = np.empty(len(starts), np.int64)
    pos = 0
    for i, L in enumerate(lens):
        room = P - (pos % P)
        if L > room:
            pos += room
        assert L <= P, f"class run of length {L} exceeds chunk size"
        place[i] = pos
        pos += L
    return place, lens, starts, pos


def _pack_core(cls_loc: np.ndarray, x_core: np.ndarray, centers_shard):
    """Pack one core's sorted items into quarter-slot-aligned chunks.
    Permutes shard quarters so the fullest quarters get the 5-chunk
    slots. Returns (device input arrays, slot->original-quarter order)."""
    xk = np.zeros((NPAD, DIM), np.float32)
    lidx = np.full(NPAD, GJUNK, np.int32)
    rank = (np.arange(NPAD) % P).astype(np.float32)
    uslot = np.full(NPAD, OOB, np.int32)
    af = np.zeros(NPAD, np.float32)

    qstart = np.searchsorted(cls_loc, np.arange(NQ + 1) * QROWS)
    packs = []
    for q in range(NQ):
        c = cls_loc[qstart[q]:qstart[q + 1]]
        packs.append(_quarter_pack(c))
    order = np.argsort([-p[3] for p in packs], kind="stable")

    # permuted shard for the gather source + bulk copies
    shard = np.empty((SHARD + 1, DIM), np.float32)
    for s, oq in enumerate(order):
        shard[s * QROWS:(s + 1) * QROWS] = \
            centers_shard[oq * QROWS:(oq + 1) * QROWS]
    shard[SHARD] = 0.0

    for s, oq in enumerate(order):
        place, lens, starts, packed = packs[oq]
        assert packed <= CAPS[s] * P, \
            f"quarter needs {packed} slots > {CAPS[s] * P}"
        lo, hi = qstart[oq], qstart[oq + 1]
        if hi == lo:
            continue
        c = cls_loc[lo:hi]
        mq = hi - lo
        base = BASE[s] * P
        out_pos = base + np.repeat(place, lens) + (
            np.arange(mq) - np.repeat(starts, lens))
        xk[out_pos] = x_core[lo:hi]
        # gather row in the PERMUTED shard
        lidx[out_pos] = (c - oq * QROWS) + s * QROWS
        rank[out_pos] = np.repeat((place % P).astype(np.float32), lens)
        uslot[base + place] = c[starts] - oq * QROWS  # quarter-local row
        af[base + place] = -ALPHA / (lens + 1.0).astype(np.float32)

    def cols(a):
        return np.ascontiguousarray(a.reshape(NCHUNK, P).T)

    return {
        "centers_s": shard,
        "x_s": xk,
        "lidx_s": cols(lidx),
        "rank_s": cols(rank.astype(np.float32)),
        "uslot_s": cols(uslot),
        "af_s": cols(af.astype(np.float32)),
    }, order


def make_in_maps(x: np.ndarray, y: np.ndarray, centers: np.ndarray):
    order = np.argsort(y, kind="stable")
    ys = y[order]
    xs = x[order]
    bounds = np.searchsorted(ys, np.arange(NCORES + 1) * SHARD)

    in_maps = []
    qorders = []
    for k in range(NCORES):
        lo, hi = bounds[k], bounds[k + 1]
        im, qorder = _pack_core(
            (ys[lo:hi] - k * SHARD).astype(np.int64), xs[lo:hi],
            centers[k * SHARD:(k + 1) * SHARD])
        in_maps.append(im)
        qorders.append(qorder)
    return in_maps, qorders


def assemble(results, qorders):
    parts = []
    for k in range(NCORES):
        inv = np.empty(NQ, np.int64)
        inv[qorders[k]] = np.arange(NQ)  # original quarter oq -> slot
        for oq in range(NQ):
            parts.append(results[k][OUT_NAMES[inv[oq]]])
    new_centers = np.concatenate(parts, axis=0)
    loss = np.float32(sum(float(results[k]["loss_s"][0, 0])
                          for k in range(NCORES)))
    return loss, new_centers


LAST_RESULTS = None


def kernel(x: np.ndarray, y: np.ndarray, centers: np.ndarray):
    global LAST_RESULTS
    x = np.ascontiguousarray(np.asarray(x, np.float32))
    y = np.asarray(y, np.int32)
    centers = np.ascontiguousarray(np.asarray(centers, np.float32))

    in_maps, qorders = make_in_maps(x, y, centers)
    nc = _get_program()
    res = bass_utils.run_bass_kernel_spmd(nc, in_maps,
                                          core_ids=list(range(NCORES)))
    LAST_RESULTS = res
    return assemble(res.results, qorders)


# revision 12
# speedup vs baseline: 1.5979x; 1.0781x over previous
"""CenterLoss kernel for Trainium2 (8 NeuronCores, Bass/Tile).

Strategy (class-sharded):
  - centers [100000,128] split into 8 shards of 12500 rows (+1 junk row).
  - Batch items routed on host to the core owning their class, sorted by
    class, packed into 128-item chunks such that no class's run crosses a
    chunk boundary (pad with junk items). All host work is integer index
    bookkeeping on y only (routing/sort/counts -> -alpha/(n+1) factors).
  - The output shard is split into 4 quarter tensors (3125 rows each):
    scatters to different quarters don't false-WAW-serialize on each
    other, and each scatter only waits for its own quarter's bulk copy.
    Chunk capacity per quarter slot is static [5,5,5,4]; the host
    permutes the shard's quarters per core (biggest item load first) so
    every quarter fits its slot. 19 chunks -> 38 SWDGE indirect ops.
  - Per core the device:
      * bulk-copies its (permuted) centers shard to the 4 output
        quarters (dominant HBM traffic), with metadata/x DMAs queued
        ahead of the copies on the HWDGE rings
      * indirect-gathers each chunk's center rows ([128,1] offsets only:
        wider offset APs mis-execute on real HW), emitted before
        anything else on GpSimd so descgen hides under the copies
      * per chunk: diff = c - x; loss row-sums via ACT square+accum;
        one-hot(first-occurrence rank) matmul on PE merges duplicate
        classes; new row = c + af * upd on DVE
      * indirect-scatters final rows. Only first-occurrence rows are
        written: duplicate/junk slots carry an out-of-bounds target and
        bounds_check drops those descriptors. Scatter emission is
        interleaved across quarters to overlap completion chains.
  - Host concatenates the 8x4 output quarters (undoing the permutation)
    and sums the 8 loss partials.
"""

import numpy as np

import concourse.bass as bass
import concourse.tile as tile
from concourse import bacc, mybir
from concourse import bass_utils

NB_CLASS = 100000
DIM = 128
BATCH = 16384
LOSS_WEIGHT = 0.01
ALPHA = 0.05

NCORES = 8
SHARD = NB_CLASS // NCORES  # 12500
NQ = 4  # output quarters per shard
QROWS = SHARD // NQ  # 3125
OOB = QROWS + 7  # scatter target for dropped (dup/junk) rows
GJUNK = SHARD  # junk row index in the gather source (full shard + 1)
P = 128  # chunk size == partitions
CAPS = [5, 5, 5, 4]  # chunks per quarter slot
BASE = [0, 5, 10, 15]
NCHUNK = sum(CAPS)  # 19
NPAD = NCHUNK * P  # 2432

FP = mybir.dt.float32
I32 = mybir.dt.int32

OUT_NAMES = [f"newq{q}_s" for q in range(NQ)]


def _build_program():
    nc = bacc.Bacc("TRN2", target_bir_lowering=False, debug=False,
                   num_devices=NCORES)

    centers_t = nc.dram_tensor("centers_s", [SHARD + 1, DIM], FP,
                               kind="ExternalInput")
    x_t = nc.dram_tensor("x_s", [NPAD, DIM], FP, kind="ExternalInput")
    lidx_t = nc.dram_tensor("lidx_s", [P, NCHUNK], I32, kind="ExternalInput")
    rank_t = nc.dram_tensor("rank_s", [P, NCHUNK], FP, kind="ExternalInput")
    uslot_t = nc.dram_tensor("uslot_s", [P, NCHUNK], I32, kind="ExternalInput")
    af_t = nc.dram_tensor("af_s", [P, NCHUNK], FP, kind="ExternalInput")

    newq_t = [nc.dram_tensor(OUT_NAMES[q], [QROWS, DIM], FP,
                             kind="ExternalOutput") for q in range(NQ)]
    loss_t = nc.dram_tensor("loss_s", [1, 1], FP, kind="ExternalOutput")

    centers_ap = centers_t.ap()

    with tile.TileContext(nc) as tc:
        with tc.tile_pool(name="const", bufs=1) as cpool, \
             tc.tile_pool(name="gat", bufs=NCHUNK) as gpool, \
             tc.tile_pool(name="out", bufs=NCHUNK) as opool, \
             tc.tile_pool(name="xs", bufs=NQ) as xpool, \
             tc.tile_pool(name="work", bufs=6) as wpool, \
             tc.tile_pool(name="psum", bufs=6, space="PSUM") as ppool, \
             tc.tile_pool(name="psl", bufs=1, space="PSUM") as plpool:

            # ---- metadata first: the gathers need lidx immediately ----
            lidx_m = cpool.tile([P, NCHUNK], I32)
            nc.sync.dma_start(lidx_m[:], lidx_t.ap()[:, :])
            uslot_m = cpool.tile([P, NCHUNK], I32)
            nc.sync.dma_start(uslot_m[:], uslot_t.ap()[:, :])
            rank_m = cpool.tile([P, NCHUNK], FP)
            nc.scalar.dma_start(rank_m[:], rank_t.ap()[:, :])
            af_m = cpool.tile([P, NCHUNK], FP)
            nc.scalar.dma_start(af_m[:], af_t.ap()[:, :])

            # ---- gathers for slots 0,2 first: keeps Q7 busy under the
            # bulk copy; slot 1,3 gathers are emitted interleaved with the
            # slot 0,2 scatters further down so Q7 never idles on the
            # per-tensor scatter completion chains ----
            cgs = [None] * NCHUNK

            def gather(c):
                cg = gpool.tile([P, DIM], FP, tag="cg", name=f"cg{c}")
                nc.gpsimd.indirect_dma_start(
                    out=cg[:],
                    out_offset=None,
                    in_=centers_ap[:, :],
                    in_offset=bass.IndirectOffsetOnAxis(
                        ap=lidx_m[:, c:c + 1], axis=0))
                cgs[c] = cg

            def alternate(sa, sb):
                out = []
                for j in range(max(CAPS[sa], CAPS[sb])):
                    if j < CAPS[sa]:
                        out.append(BASE[sa] + j)
                    if j < CAPS[sb]:
                        out.append(BASE[sb] + j)
                return out

            early = alternate(0, 2)
            late = alternate(1, 3)
            for c in early:
                gather(c)

            # ---- x loads, then bulk copies, interleaved on both rings ----
            # ring order (FIFO per issuing engine):
            #   sync:   lidx, uslot, xg0, copy s0, xg1, copy s1
            #   scalar: rank, af,   xg2, copy s2, xg3, copy s3
            xgs = [None] * NQ

            def load_xg(s, eng):
                xg = xpool.tile([P, CAPS[s], DIM], FP, tag="xg", name=f"xg{s}")
                eng.dma_start(
                    xg[:],
                    x_t.ap()[BASE[s] * P:(BASE[s] + CAPS[s]) * P, :]
                       .rearrange("(c p) d -> p c d", p=P))
                xgs[s] = xg

            def copy_q(q, eng):
                eng.dma_start(newq_t[q].ap()[:, :],
                              centers_ap[q * QROWS:(q + 1) * QROWS, :])

            load_xg(0, nc.sync)
            copy_q(0, nc.sync)
            load_xg(2, nc.scalar)
            copy_q(2, nc.scalar)
            load_xg(1, nc.sync)
            copy_q(1, nc.sync)
            load_xg(3, nc.scalar)
            copy_q(3, nc.scalar)

            # ---- constants for compute ----
            iota_i = cpool.tile([P, P], I32)
            nc.gpsimd.iota(iota_i[:], pattern=[[1, P]], base=0,
                           channel_multiplier=0)
            iota_f = cpool.tile([P, P], FP)
            nc.vector.tensor_copy(iota_f[:], iota_i[:])
            ones_col = cpool.tile([P, 1], FP)
            nc.vector.memset(ones_col[:], 1.0)
            sacc_all = cpool.tile([P, NCHUNK], FP)

            # ---- per-chunk compute ----
            def slot_of(c):
                for s in range(NQ):
                    if c < BASE[s] + CAPS[s]:
                        return s
                raise ValueError(c)

            outcs = [None] * NCHUNK

            def compute(c):
                cg = cgs[c]
                s = slot_of(c)
                xg = xgs[s]
                cc = c - BASE[s]

                diff = wpool.tile([P, DIM], FP, tag="diff", name=f"diff{c}")
                nc.vector.tensor_sub(diff[:], cg[:], xg[:, cc, :])

                # loss: ACT square + free-axis accumulate into column c
                sq = wpool.tile([P, DIM], FP, tag="sq", name=f"sq{c}")
                nc.scalar.activation(
                    out=sq[:], in_=diff[:],
                    func=mybir.ActivationFunctionType.Square,
                    accum_out=sacc_all[:, c:c + 1])

                # one-hot of first-occurrence rank
                onehot = wpool.tile([P, P], FP, tag="onehot", name=f"oh{c}")
                nc.vector.tensor_tensor(
                    out=onehot[:],
                    in0=rank_m[:, c:c + 1].to_broadcast([P, P]),
                    in1=iota_f[:],
                    op=mybir.AluOpType.is_equal)

                # upd[slot, :] = sum of diff rows sharing the slot's class
                ps = ppool.tile([P, DIM], FP, tag="ps", name=f"ps{c}")
                nc.tensor.matmul(out=ps[:], lhsT=onehot[:], rhs=diff[:],
                                 start=True, stop=True)

                # new row = c + af * upd   (af = -alpha/(count+1))
                outc = opool.tile([P, DIM], FP, tag="outc", name=f"outc{c}")
                nc.vector.tensor_scalar(
                    out=outc[:], in0=ps[:],
                    scalar1=af_m[:, c:c + 1], scalar2=None,
                    op0=mybir.AluOpType.mult)
                nc.vector.tensor_add(outc[:], outc[:], cg[:])
                outcs[c] = outc

            for c in early:
                compute(c)

            # ---- scatters: slots 0,2 (whose copies finish first)
            # interleaved with the slot 1,3 gathers; then compute 1,3;
            # then the remaining scatters. Dup/junk rows carry an OOB
            # target and are dropped by the bounds check. ----
            def scatter(c):
                s = slot_of(c)
                # bounds_check + oob_is_err=False is required anyway: the
                # no-bounds-regs indirect-scatter ucode wedges on HW.
                nc.gpsimd.indirect_dma_start(
                    out=newq_t[s].ap()[:, :],
                    out_offset=bass.IndirectOffsetOnAxis(
                        ap=uslot_m[:, c:c + 1], axis=0),
                    in_=outcs[c][:],
                    in_offset=None,
                    bounds_check=QROWS - 1,
                    oob_is_err=False)

            for j in range(CAPS[0] + CAPS[2]):
                scatter(early[j])
                if j < len(late):
                    gather(late[j])
            for c in late:
                compute(c)
            for c in late:
                scatter(c)

            # ---- loss: reduce columns, cross-partition sum, scale ----
            lacc = cpool.tile([P, 1], FP)
            nc.vector.tensor_reduce(out=lacc[:], in_=sacc_all[:],
                                    axis=mybir.AxisListType.X,
                                    op=mybir.AluOpType.add)
            psl = plpool.tile([1, 1], FP)
            nc.tensor.matmul(out=psl[:], lhsT=lacc[:], rhs=ones_col[:],
                             start=True, stop=True)
            loss_sb = cpool.tile([1, 1], FP)
            nc.vector.tensor_scalar_mul(loss_sb[:], psl[:],
                                        LOSS_WEIGHT / BATCH)
            nc.sync.dma_start(loss_t.ap()[:, :], loss_sb[:])

    nc.compile()
    return nc


_NC = None


def _get_program():
    global _NC
    if _NC is None:
        _NC = _build_program()
    return _NC


def _quarter_pack(c: np.ndarray):
    """Greedy chunk packing of one quarter's sorted local classes.
    Returns (place, lens, starts, packed_size)."""
    mq = c.shape[0]
    if mq == 0:
        z = np.zeros(0, np.int64)
        return z, z, z, 0
    starts = np.flatnonzero(np.r_[True, c[1:] != c[:-1]])
    lens = np.diff(np.r_[starts, mq])
    place = np.empty(len(starts), np.int64)
    pos = 0
    for i, L in enumerate(lens):
        room = P - (pos % P)
        if L > room:
            pos += room
        assert L <= P, f"class run of length {L} exceeds chunk size"
        place[i] = pos
        pos += L
    return place, lens, starts, pos


def _pack_core(cls_loc: np.ndarray, x_core: np.ndarray, centers_shard):
    """Pack one core's sorted items into quarter-slot-aligned chunks.
    Permutes shard quarters so the fullest quarters get the 5-chunk
    slots. Returns (device input arrays, slot->original-quarter order)."""
    xk = np.zeros((NPAD, DIM), np.float32)
    lidx = np.full(NPAD, GJUNK, np.int32)
    rank = (np.arange(NPAD) % P).astype(np.float32)
    uslot = np.full(NPAD, OOB, np.int32)
    af = np.zeros(NPAD, np.float32)

    qstart = np.searchsorted(cls_loc, np.arange(NQ + 1) * QROWS)
    packs = []
    for q in range(NQ):
        c = cls_loc[qstart[q]:qstart[q + 1]]
        packs.append(_quarter_pack(c))
    order = np.argsort([-p[3] for p in packs], kind="stable")

    # permuted shard for the gather source + bulk copies
    shard = np.empty((SHARD + 1, DIM), np.float32)
    for s, oq in enumerate(order):
        shard[s * QROWS:(s + 1) * QROWS] = \
            centers_shard[oq * QROWS:(oq + 1) * QROWS]
    shard[SHARD] = 0.0

    for s, oq in enumerate(order):
        place, lens, starts, packed = packs[oq]
        assert packed <= CAPS[s] * P, \
            f"quarter needs {packed} slots > {CAPS[s] * P}"
        lo, hi = qstart[oq], qstart[oq + 1]
        if hi == lo:
            continue
        c = cls_loc[lo:hi]
        mq = hi - lo
        base = BASE[s] * P
        out_pos = base + np.repeat(place, lens) + (
            np.arange(mq) - np.repeat(starts, lens))
        xk[out_pos] = x_core[lo:hi]
        # gather row in the PERMUTED shard
        lidx[out_pos] = (c - oq * QROWS) + s * QROWS
        rank[out_pos] = np.repeat((place % P).astype(np.float32), lens)
        uslot[base + place] = c[starts] - oq * QROWS  # quarter-local row
        af[base + place] = -ALPHA / (lens + 1.0).astype(np.float32)

    def cols(a):
        return np.ascontiguousarray(a.reshape(NCHUNK, P).T)

    return {
        "centers_s": shard,
        "x_s": xk,
        "lidx_s": cols(lidx),
        "rank_s": cols(rank.astype(np.float32)),
        "uslot_s": cols(uslot),
        "af_s": cols(af.astype(np.float32)),
    }, order


def make_in_maps(x: np.ndarray, y: np.ndarray, centers: np.ndarray):
    order = np.argsort(y, kind="stable")
    ys = y[order]
    xs = x[order]
    bounds = np.searchsorted(ys, np.arange(NCORES + 1) * SHARD)

    in_maps = []
    qorders = []
    for k in range(NCORES):
        lo, hi = bounds[k], bounds[k + 1]
        im, qorder = _pack_core(
            (ys[lo:hi] - k * SHARD).astype(np.int64), xs[lo:hi],
            centers[k * SHARD:(k + 1) * SHARD])
        in_maps.append(im)
        qorders.append(qorder)
    return in_maps, qorders


def assemble(results, qorders):
    parts = []
    for k in range(NCORES):
        inv = np.empty(NQ, np.int64)
        inv[qorders[k]] = np.arange(NQ)  # original quarter oq -> slot
        for oq in range(NQ):
            parts.append(results[k][OUT_NAMES[inv[oq]]])
    new_centers = np.concatenate(parts, axis=0)
    loss = np.float32(sum(float(results[k]["loss_s"][0, 0])
                          for k in range(NCORES)))
    return loss, new_centers


LAST_RESULTS = None


def kernel(x: np.ndarray, y: np.ndarray, centers: np.ndarray):
    global LAST_RESULTS
    x = np.ascontiguousarray(np.asarray(x, np.float32))
    y = np.asarray(y, np.int32)
    centers = np.ascontiguousarray(np.asarray(centers, np.float32))

    in_maps, qorders = make_in_maps(x, y, centers)
    nc = _get_program()
    res = bass_utils.run_bass_kernel_spmd(nc, in_maps,
                                          core_ids=list(range(NCORES)))
    LAST_RESULTS = res
    return assemble(res.results, qorders)
